# revision 1
# baseline (speedup 1.0000x reference)
"""Trainium2 Bass kernel for nn_DeformableAlignment.

Sharding: 8 cores = (batch b in 0..4) x (image row-half in {0,1}).
Each core computes out[b, :, y0:y0+64, :] for y0 = 64*(i%2).

Math (per core, matches reference exactly):
  om  = conv3x3(concat(f1,f3))                          [27, 64, 128]
  dy/dx per tap k; sg = sigmoid(mask-channels)
  bilinear warp written floor-free via hat fields:
    cym[k,sy] = relu(1-|dy-sy|)*sg  (sy in -2..2)       y-coeffs (mask folded)
    cx [k,sx] = relu(1-|dx-sx|)                         x-coeffs
  g[k] = 1x1-conv of f1 with main_w tap k               [o, y', x]
  V[k] = sum_sy cym[k,sy] * g[k] shifted in y           (free-dim y shifts)
  out  = sum_k sum_sx cx[k,sx] * V[k] shifted in x      (free-dim x shifts,
                                                         after PE transpose)
  BN stats via on-device partial sums + AllReduce across 8 cores.

Layouts:
  stage V: [x=128 partitions, (o64, y64) free]
  stage H: [(o-parity, y64)=128 partitions, (o-pair j32, x128) free]
Out-of-image samples contribute zero via zero-padded f1/x windows.
"""

import numpy as np
import ml_dtypes

import concourse.bass as bass
import concourse.bacc as bacc
import concourse.tile as tile
from concourse import mybir
from concourse.bass_utils import run_bass_kernel_spmd

f32 = mybir.dt.float32
bf16 = mybir.dt.bfloat16
AF = mybir.ActivationFunctionType
OP = mybir.AluOpType

N_CORES = 8
SY = [-2, -1, 0, 1, 2]
SX = [-2, -1, 0, 1, 2]
NSY = len(SY)
NSX = len(SX)
EPS = 1e-5
BN_N = 4 * 128 * 128  # elements per channel for batch stats


def bcast(ap, n, dim):
    """Insert a broadcast (step-0) dim of size n at position dim (free dims)."""
    new = [list(p) for p in ap.ap]
    new.insert(dim, [0, n])
    return bass.AP(tensor=ap.tensor, offset=ap.offset, ap=new)


def build_module(debug=False):
    nc = bacc.Bacc("TRN2", target_bir_lowering=False, debug=False,
                   num_devices=N_CORES)
    xcat_d = nc.dram_tensor("xcat", [128, 66, 130], bf16, kind="ExternalInput")
    f1s_d = nc.dram_tensor("f1s", [64, 70, 134], bf16, kind="ExternalInput")
    ow_d = nc.dram_tensor("ow", [128, 9, 27], bf16, kind="ExternalInput")
    wk_d = nc.dram_tensor("wk", [64, 9, 64], bf16, kind="ExternalInput")
    id_d = nc.dram_tensor("ident", [128, 128], bf16, kind="ExternalInput")
    sel_d = nc.dram_tensor("sel", [128, 2], f32, kind="ExternalInput")
    ob_d = nc.dram_tensor("ob", [27, 1], f32, kind="ExternalInput")
    gb_d = nc.dram_tensor("gb", [2, 2, 32], f32, kind="ExternalInput")
    out_d = nc.dram_tensor("out", [64, 64, 128], f32, kind="ExternalOutput")
    dbg = {}
    if debug:
        dbg["omT"] = nc.dram_tensor("d_omT", [128, 64, 27], bf16,
                                    kind="ExternalOutput")
        dbg["cym"] = nc.dram_tensor("d_cym", [128, 9, NSY, 64], bf16,
                                    kind="ExternalOutput")
        dbg["cx2"] = nc.dram_tensor("d_cx2", [128, 9, NSX, 64], bf16,
                                    kind="ExternalOutput")
        dbg["g0"] = nc.dram_tensor("d_g0", [128, 3, 64, 70], bf16,
                                   kind="ExternalOutput")
        dbg["hacc"] = nc.dram_tensor("d_hacc", [128, 32, 128], bf16,
                                     kind="ExternalOutput")

    cp_engines = None

    def cp(out, in_):
        # round-robin copies across DVE / ACT / GPSIMD
        eng = next(cp_engines)
        if eng == 0:
            nc.vector.tensor_copy(out, in_)
        elif eng == 1:
            nc.scalar.copy(out, in_)
        else:
            nc.gpsimd.tensor_copy(out, in_)

    import itertools
    cp_engines = itertools.cycle([0, 1])

    with tile.TileContext(nc) as tc:
        import contextlib
        ctx = contextlib.ExitStack()
        with ctx:
            const = ctx.enter_context(tc.tile_pool(name="const", bufs=1))
            xband = ctx.enter_context(tc.tile_pool(name="xband", bufs=3))
            omchp = ctx.enter_context(tc.tile_pool(name="omch", bufs=2))
            fldp = ctx.enter_context(tc.tile_pool(name="fld", bufs=1))
            gpool = ctx.enter_context(tc.tile_pool(name="g", bufs=2))
            warp = ctx.enter_context(tc.tile_pool(name="warp", bufs=3))
            vtp = ctx.enter_context(tc.tile_pool(name="vt", bufs=2))
            finp = ctx.enter_context(tc.tile_pool(name="fin", bufs=3))
            dram = ctx.enter_context(tc.tile_pool(name="dram", bufs=1,
                                                  space="DRAM"))
            phase1 = contextlib.ExitStack()
            pom = phase1.enter_context(tc.tile_pool(name="pom", bufs=2,
                                                    space="PSUM"))
            ptr = phase1.enter_context(tc.tile_pool(name="ptr", bufs=2,
                                                    space="PSUM"))

            # ---- constants in ----
            ow_sb = const.tile([128, 9, 27], bf16)
            nc.sync.dma_start(out=ow_sb, in_=ow_d[:])
            wk_sb = const.tile([64, 9, 64], bf16)
            nc.sync.dma_start(out=wk_sb, in_=wk_d[:])
            ident = const.tile([128, 128], bf16)
            nc.sync.dma_start(out=ident, in_=id_d[:])
            sel = const.tile([128, 2], f32)
            nc.sync.dma_start(out=sel, in_=sel_d[:])
            ob_sb = const.tile([27, 1], f32)
            nc.sync.dma_start(out=ob_sb, in_=ob_d[:])
            f1s_sb = const.tile([64, 70, 134], bf16)
            nc.sync.dma_start(out=f1s_sb, in_=f1s_d[:])
            syc = const.tile([128, NSY, 64], bf16)
            sxc = const.tile([128, NSX, 64], bf16)
            for i, s in enumerate(SY):
                nc.vector.memset(syc[:, i, :], float(s))
            for i, s in enumerate(SX):
                nc.vector.memset(sxc[:, i, :], float(s))

            # ---- offset conv + transpose to om_T [x, y, 27] ----
            om_T = fldp.tile([128, 64, 27], bf16)
            for c in range(16):  # chunks of 4 output rows
                band = xband.tile([128, 6, 130], bf16)
                nc.sync.dma_start(out=band, in_=xcat_d[:, 4 * c:4 * c + 6, :])
                ps = pom.tile([27, 512], f32)
                for k in range(9):
                    ky, kx = k // 3, k % 3
                    rhs = band[:, ky:ky + 4, kx:kx + 128]
                    nc.tensor.matmul(ps, ow_sb[:, k, :], rhs,
                                     start=(k == 0), stop=(k == 8))
                om_ch = omchp.tile([27, 4, 128], bf16)
                nc.vector.tensor_scalar(
                    om_ch, ps.rearrange("p (y x) -> p y x", y=4),
                    ob_sb, None, OP.add)
                pt = ptr.tile([128, 4, 28], bf16)
                for j in range(4):
                    nc.tensor.transpose(pt[:, j, 0:27], om_ch[:, j, :],
                                        ident[0:27, 0:27])
                cp(om_T[:, 4 * c:4 * c + 4, :], pt[:, :, 0:27])
            if debug:
                nc.sync.dma_start(out=dbg["omT"][:], in_=om_T)

            # ---- y-direction fields: cym [x, k, sy, y] ----
            sg = fldp.tile([128, 9, 64], bf16)
            nc.scalar.activation(
                sg, om_T[:, :, 18:27].rearrange("x y k -> x k y"), AF.Sigmoid)
            dyp = fldp.tile([128, 9, 64], bf16)
            nc.vector.tensor_copy(
                dyp, om_T[:, :, 0:18:2].rearrange("x y k -> x k y"))
            ty = fldp.tile([128, 9, NSY, 64], bf16)
            nc.vector.tensor_tensor(
                out=ty, in0=bcast(dyp, NSY, 2), in1=bcast(syc, 9, 1),
                op=OP.subtract)
            nc.scalar.activation(ty, ty, AF.Abs)
            nc.vector.tensor_scalar(ty, ty, -1.0, 1.0, OP.mult, OP.add)
            nc.vector.tensor_scalar(ty, ty, 0.0, None, OP.max)
            cym = fldp.tile([128, 9, NSY, 64], bf16)
            nc.vector.tensor_tensor(out=cym, in0=ty, in1=bcast(sg, NSY, 2),
                                    op=OP.mult)
            if debug:
                nc.sync.dma_start(out=dbg["cym"][:], in_=cym)

            # ---- x-direction fields in x-part layout: cxP [x, k, sx, y] ----
            dxp = fldp.tile([128, 9, 64], bf16)
            nc.vector.tensor_copy(
                dxp, om_T[:, :, 1:18:2].rearrange("x y k -> x k y"))
            tx = fldp.tile([128, 9, NSX, 64], bf16)
            nc.vector.tensor_tensor(
                out=tx, in0=bcast(dxp, NSX, 2), in1=bcast(sxc, 9, 1),
                op=OP.subtract)
            nc.scalar.activation(tx, tx, AF.Abs)
            nc.vector.tensor_scalar(tx, tx, -1.0, 1.0, OP.mult, OP.add)
            nc.vector.tensor_scalar(tx, tx, 0.0, None, OP.max)
            cxP = tx
            # B fields: Bf[x, k, sx, sy, y] = cxP * cym
            Bf = fldp.tile([128, 9, NSX, NSY, 64], bf16)
            nc.vector.tensor_tensor(
                out=Bf, in0=bcast(cxP, NSY, 3), in1=bcast(cym, NSX, 2),
                op=OP.mult)
            if debug:
                nc.sync.dma_start(out=dbg["cx2"][:], in_=cxP)

            # ---- main loop over ky-groups ----
            phase1.close()
            pg = ctx.enter_context(tc.tile_pool(name="pg", bufs=2,
                                                space="PSUM"))
            pv = ctx.enter_context(tc.tile_pool(name="pv", bufs=2,
                                                space="PSUM"))
            pst = ctx.enter_context(tc.tile_pool(name="pst", bufs=2,
                                                 space="PSUM"))
            acc = warp.tile([128, 64, 64], bf16, tag="acc", bufs=1)
            first_term = True
            VMIN = min(kx - 1 + s for kx in range(3) for s in SX)
            VMAX = max(kx - 1 + s for kx in range(3) for s in SX)
            for kg in range(3):
                for v in range(VMIN, VMAX + 1):
                    kls = [kl for kl in range(3) if (v - (kl - 1)) in SX]
                    if not kls:
                        continue
                    g_v = gpool.tile([128, 3, 64, 70], bf16, tag="g")
                    for rb in range(0, 70, 4):
                        nrow = min(4, 70 - rb)
                        psg = pg.tile([128, 4, 256], f32)
                        for j in range(nrow):
                            nc.tensor.matmul(
                                psg[:, j, 0:192],
                                f1s_sb[:, rb + j, 3 + v:3 + v + 128],
                                wk_sb[:, 3 * kg:3 * kg + 3, :].rearrange(
                                    "c k o -> c (k o)"),
                                start=True, stop=True)
                        cp(g_v[:, :, :, rb:rb + nrow],
                           psg[:, 0:nrow, 0:192].rearrange(
                               "x j (k o) -> x k o j", k=3))
                    for kl in kls:
                        k = 3 * kg + kl
                        sxi = SX.index(v - (kl - 1))
                        for syi, sy in enumerate(SY):
                            off = kg - 1 + sy + 3
                            in0 = g_v[:, kl, :, off:off + 64]
                            in1 = bcast(Bf[:, k, sxi, syi, :], 64, 1)
                            if first_term:
                                nc.vector.tensor_tensor(
                                    out=acc, in0=in0, in1=in1, op=OP.mult)
                                first_term = False
                            else:
                                tmp = warp.tile([128, 64, 64], bf16,
                                                tag="wtmp")
                                nc.vector.tensor_tensor(
                                    out=tmp, in0=in0, in1=in1, op=OP.mult)
                                nc.vector.tensor_tensor(
                                    out=acc, in0=acc, in1=tmp, op=OP.add)
            # transpose acc -> hacc [(par,y), j, x]
            hacc = warp.tile([128, 32, 128], bf16, tag="hacc", bufs=1)
            for j2 in range(4):
                pvt = pv.tile([128, 8, 128], bf16)
                for jj in range(8):
                    j = 8 * j2 + jj
                    nc.tensor.transpose(
                        pvt[:, jj, :],
                        acc[:, 2 * j:2 * j + 2, :].rearrange(
                            "x o y -> x (o y)"),
                        ident)
                cp(hacc[:, 8 * j2:8 * j2 + 8, :], pvt)
            if debug:
                nc.sync.dma_start(out=dbg["hacc"][:], in_=hacc)

            # ---- BN stats ----
            sq = warp.tile([128, 32, 128], bf16, tag="wtmp")
            nc.vector.tensor_tensor(out=sq, in0=hacc, in1=hacc, op=OP.mult)
            stat2 = fldp.tile([128, 2, 32], f32)
            nc.vector.tensor_reduce(stat2[:, 0, :], hacc,
                                    axis=mybir.AxisListType.X, op=OP.add)
            nc.vector.tensor_reduce(stat2[:, 1, :], sq,
                                    axis=mybir.AxisListType.X, op=OP.add)
            ps1 = pst.tile([2, 2, 32], f32)
            nc.tensor.matmul(ps1.rearrange("p a b -> p (a b)"), sel,
                             stat2.rearrange("p a b -> p (a b)"),
                             start=True, stop=True)
            st_sb = fldp.tile([2, 2, 32], f32)
            nc.vector.tensor_copy(st_sb, ps1)
            cc_in = dram.tile([2, 2, 32], f32)
            cc_out = dram.tile([2, 2, 32], f32)
            nc.sync.dma_start(out=cc_in[:], in_=st_sb)
            nc.gpsimd.collective_compute(
                "AllReduce", OP.add,
                replica_groups=[list(range(N_CORES))],
                ins=[cc_in[:]], outs=[cc_out[:]])
            red = fldp.tile([2, 2, 32], f32)
            nc.sync.dma_start(out=red, in_=cc_out[:])

            gb_sb = fldp.tile([2, 2, 32], f32)
            nc.sync.dma_start(out=gb_sb, in_=gb_d[:])
            mt = fldp.tile([2, 32], f32)
            nc.vector.tensor_scalar(mt, red[:, 0, :], 1.0 / BN_N, None,
                                    OP.mult)
            ex2 = fldp.tile([2, 32], f32)
            nc.vector.tensor_scalar(ex2, red[:, 1, :], 1.0 / BN_N, None,
                                    OP.mult)
            var = fldp.tile([2, 32], f32)
            nc.vector.tensor_tensor(out=var, in0=mt, in1=mt, op=OP.mult)
            nc.vector.tensor_tensor(out=var, in0=ex2, in1=var, op=OP.subtract)
            nc.vector.tensor_scalar(var, var, EPS, None, OP.add)
            sqv = fldp.tile([2, 32], f32)
            nc.scalar.activation(sqv, var, AF.Sqrt)
            rstd = fldp.tile([2, 32], f32)
            nc.vector.reciprocal(rstd, sqv)
            AB = fldp.tile([2, 2, 32], f32)
            nc.vector.tensor_tensor(out=AB[:, 0, :], in0=gb_sb[:, 0, :],
                                    in1=rstd, op=OP.mult)
            nc.vector.tensor_tensor(out=AB[:, 1, :], in0=mt, in1=AB[:, 0, :],
                                    op=OP.mult)
            nc.vector.tensor_tensor(out=AB[:, 1, :], in0=gb_sb[:, 1, :],
                                    in1=AB[:, 1, :], op=OP.subtract)
            ab_d = dram.tile([2, 2, 32], f32)
            nc.sync.dma_start(out=ab_d[:], in_=AB)
            ABc = fldp.tile([128, 2, 32], f32)
            nc.sync.dma_start(
                out=ABc,
                in_=bass.AP(tensor=ab_d.tensor, offset=ab_d.offset,
                            ap=[[64, 2], [0, 64], [32, 2], [1, 32]]))

            # ---- BN apply + store ----
            for j in range(32):
                fin = finp.tile([128, 128], f32)
                nc.vector.tensor_scalar(fin, hacc[:, j, :],
                                        ABc[:, 0, j:j + 1],
                                        ABc[:, 1, j:j + 1],
                                        OP.mult, OP.add)
                nc.sync.dma_start(
                    out=out_d[2 * j:2 * j + 2, :, :], in_=fin)

    nc.finalize()
    return nc


_module_cache = {}


def get_module(debug=False):
    key = bool(debug)
    if key not in _module_cache:
        _module_cache[key] = build_module(debug)
    return _module_cache[key]


def prep_inputs(f1_feat, f3_feat, offset_w, offset_b, main_w, gamma, beta):
    """Host-side slicing/padding; returns list of 8 in_maps."""
    bf = ml_dtypes.bfloat16
    f1 = np.asarray(f1_feat, np.float32)
    f3 = np.asarray(f3_feat, np.float32)
    ow = np.asarray(offset_w, np.float32)   # [27,128,3,3]
    ob = np.asarray(offset_b, np.float32).reshape(27, 1)
    wk = np.asarray(main_w, np.float32)     # [64,64,3,3]

    cat = np.concatenate([f1, f3], axis=1)  # [4,128,128,128]
    # ow_t[c, k, m] = ow[m, c, ky, kx]
    ow_t = ow.reshape(27, 128, 9).transpose(1, 2, 0).copy().astype(bf)
    wk_t = wk.reshape(64, 64, 9).transpose(1, 2, 0).copy().astype(bf)
    ident = np.eye(128, dtype=np.float32).astype(bf)
    sel = np.zeros((128, 2), np.float32)
    sel[0:64, 0] = 1.0
    sel[64:128, 1] = 1.0
    gb = np.stack([np.asarray(gamma, np.float32).reshape(2, 32),
                   np.asarray(beta, np.float32).reshape(2, 32)], axis=1)
    # wait: gb layout [2(par), 2(g/b), 32]: gamma[o] -> (par, pair): o=2*pair+par
    gam = np.asarray(gamma, np.float32)
    bet = np.asarray(beta, np.float32)
    gb = np.zeros((2, 2, 32), np.float32)
    for par in range(2):
        gb[par, 0, :] = gam[par::2]
        gb[par, 1, :] = bet[par::2]

    maps = []
    for i in range(N_CORES):
        b, half = i // 2, i % 2
        y0 = 64 * half
        xc = np.zeros((128, 66, 130), np.float32)
        lo, hi = max(0, y0 - 1), min(128, y0 + 65)
        xc[:, lo - (y0 - 1):hi - (y0 - 1), 1:129] = cat[b][:, lo:hi, :]
        f1s = np.zeros((64, 70, 134), np.float32)
        lo2, hi2 = max(0, y0 - 3), min(128, y0 + 67)
        f1s[:, lo2 - (y0 - 3):hi2 - (y0 - 3), 3:131] = f1[b][:, lo2:hi2, :]
        maps.append({
            "xcat": xc.astype(bf), "f1s": f1s.astype(bf),
            "ow": ow_t, "wk": wk_t, "ident": ident, "sel": sel, "gb": gb,
            "ob": ob,
        })
    return maps


def kernel(**inputs):
    nc = get_module(debug=False)
    maps = prep_inputs(**inputs)
    res = run_bass_kernel_spmd(nc, maps, core_ids=list(range(N_CORES)))
    out = np.zeros((4, 64, 128, 128), np.float32)
    for i in range(N_CORES):
        b, half = i // 2, i % 2
        # device out: [o(pair-major), y, x] with o = 2*j + par ordering:
        # out_d rows 2j..2j+1 hold (par=0, par=1) for pair j -> o = 2j+par
        dev = res.results[i]["out"]                 # [64, 64, 128]
        o_order = np.arange(64).reshape(32, 2).reshape(-1)  # identity
        out[b, :, 64 * half:64 * half + 64, :] = dev
    return out


if __name__ == "__main__":
    d = np.load("/root/problem/ref_cache.npz")
    inp = {k: d[k] for k in d.files if k != "expected"}
    got = kernel(**inp)
    exp = d["expected"]
    err = np.linalg.norm(got - exp) / np.linalg.norm(exp)
    print("rel l2 err:", err, "maxabs:", np.abs(got - exp).max())



# revision 24
# speedup vs baseline: 2.0283x; 2.0283x over previous
"""Trainium2 Bass kernel for nn_DeformableAlignment.

Sharding: 8 cores = (batch b in 0..4) x (image row-half in {0,1}).
Each core computes out[b, :, y0:y0+64, :] for y0 = 64*(i%2).

Math (per core, matches reference exactly):
  om  = conv3x3(concat(f1,f3))                          [27, 64, 128]
  dy/dx per tap k; sg = sigmoid(mask-channels)
  bilinear warp written floor-free via hat fields:
    cym[k,sy] = relu(1-|dy-sy|)*sg  (sy in -2..2)       y-coeffs (mask folded)
    cx [k,sx] = relu(1-|dx-sx|)                         x-coeffs
  g[k] = 1x1-conv of f1 with main_w tap k, computed ONCE on the
         y-padded grid: g[x=128 part, (k,o), y70]       (140 matmuls)
  V[k] = sum_sy cym[k,sy] * g[k] shifted in y           (free-dim y offsets)
  out  = sum_k sum_sx cx[k,sx] * V[k] shifted in x      (x-shifts = 6
         partition-shifted SBUF->SBUF DMAs of V)
  BN stats via on-device partial sums + AllReduce across 8 cores.

IO is minimized for the axon tunnel (upload ~94MB/s, download ~40MB/s):
  feat  [128, 70, 128] bf16: parts 0-63 f1 rows y0-3..y0+66,
                             parts 64-127 f3 rows y0-1..y0+64 (+2 pad rows)
  wpack [128, 659] bf16: ow_t [128,243] | wk packed [128,288] | ident
  spack [128, 4] f32: sel | sel | ob | gb-flat
  out   [64, 64, 128] bf16 (converted to f32 on host)
"""

import numpy as np
import ml_dtypes

import concourse.bass as bass
import concourse.bacc as bacc
import concourse.tile as tile
from concourse import mybir
from concourse.bass_utils import run_bass_kernel_spmd

f32 = mybir.dt.float32
bf16 = mybir.dt.bfloat16
AF = mybir.ActivationFunctionType
OP = mybir.AluOpType

N_CORES = 8
NS = 5  # shifts -2..2
EPS = 1e-5
BN_N = 4 * 128 * 128  # elements per channel for batch stats


def bcast(ap, n, dim):
    """Insert a broadcast (step-0) dim of size n at position dim."""
    new = [list(p) for p in ap.ap]
    new.insert(dim, [0, n])
    return bass.AP(tensor=ap.tensor, offset=ap.offset, ap=new)


def build_module():
    nc = bacc.Bacc("TRN2", target_bir_lowering=False, debug=False,
                   num_devices=N_CORES)
    feat_d = nc.dram_tensor("feat", [128, 70, 128], bf16, kind="ExternalInput")
    wp_d = nc.dram_tensor("wpack", [128, 659], bf16, kind="ExternalInput")
    sp_d = nc.dram_tensor("spack", [128, 4], f32, kind="ExternalInput")
    out_d = nc.dram_tensor("out", [64, 64, 128], bf16, kind="ExternalOutput")

    import itertools
    cp_engines = itertools.cycle([0, 1])

    def cp(out, in_):
        if next(cp_engines) == 0:
            nc.vector.tensor_copy(out, in_)
        else:
            nc.scalar.copy(out, in_)

    with tile.TileContext(nc) as tc:
        import contextlib
        ctx = contextlib.ExitStack()
        with ctx:
            const = ctx.enter_context(tc.tile_pool(name="const", bufs=1))
            fld = ctx.enter_context(tc.tile_pool(name="fld", bufs=1))
            dram = ctx.enter_context(tc.tile_pool(name="dram", bufs=1,
                                                  space="DRAM"))
            tmpp = ctx.enter_context(tc.tile_pool(name="tmpp", bufs=1))
            outp = ctx.enter_context(tc.tile_pool(name="outp", bufs=1))
            bfp = ctx.enter_context(tc.tile_pool(name="bfp", bufs=1))

            # ---- constants in ----
            ow_sb = const.tile([128, 9, 27], bf16)
            nc.sync.dma_start(out=ow_sb, in_=wp_d[:, 0:243])
            wk_sb = const.tile([64, 576], bf16)
            nc.sync.dma_start(out=wk_sb[:, 0:288], in_=wp_d[0:64, 243:531])
            nc.sync.dma_start(out=wk_sb[:, 288:576], in_=wp_d[64:128, 243:531])
            ident = const.tile([128, 128], bf16)
            nc.sync.dma_start(out=ident, in_=wp_d[:, 531:659])
            sp_sb = const.tile([128, 4], f32)
            nc.sync.dma_start(out=sp_sb, in_=sp_d[:])
            gb_sb = const.tile([2, 2, 32], f32)
            nc.sync.dma_start(out=gb_sb, in_=sp_d[:, 3:4])
            syc = const.tile([128, NS, 64], bf16)
            sxc = const.tile([128, NS, 64], bf16)
            for i in range(NS):
                nc.vector.memset(syc[:, i, :], float(i - 2))
                nc.vector.memset(sxc[:, i, :], float(i - 2))

            # ---- phase 1: offset conv + fields ----
            phase1 = contextlib.ExitStack()
            xcp = phase1.enter_context(tc.tile_pool(name="xcp", bufs=1))
            omp = phase1.enter_context(tc.tile_pool(name="omp", bufs=2))
            pom = phase1.enter_context(tc.tile_pool(name="pom", bufs=2,
                                                    space="PSUM"))
            ptr = phase1.enter_context(tc.tile_pool(name="ptr", bufs=2,
                                                    space="PSUM"))

            xcat = xcp.tile([128, 66, 130], bf16)
            nc.vector.memset(xcat, 0.0)
            nc.sync.dma_start(out=xcat[0:64, :, 1:129], in_=feat_d[0:64, 2:68, :])
            nc.sync.dma_start(out=xcat[64:128, :, 1:129],
                              in_=feat_d[64:128, 0:66, :])

            om_T = fld.tile([128, 64, 27], bf16, tag="omT")
            for c in range(16):  # chunks of 4 output rows
                ps = pom.tile([27, 512], f32)
                for k in range(9):
                    ky, kx = k // 3, k % 3
                    rhs = xcat[:, 4 * c + ky:4 * c + ky + 4, kx:kx + 128]
                    nc.tensor.matmul(ps, ow_sb[:, k, :], rhs,
                                     start=(k == 0), stop=(k == 8))
                om_ch = omp.tile([27, 4, 128], bf16)
                nc.vector.tensor_scalar(
                    om_ch, ps.rearrange("p (y x) -> p y x", y=4),
                    sp_sb[0:27, 2:3], None, OP.add)
                pt = ptr.tile([128, 4, 28], bf16)
                for j in range(4):
                    nc.tensor.transpose(pt[:, j, 0:27], om_ch[:, j, :],
                                        ident[0:27, 0:27])
                cp(om_T[:, 4 * c:4 * c + 4, :], pt[:, :, 0:27])

            # fields: cym [x, 9, 5, 64] (mask folded), cx [x, 9, 5, 64]
            sg = fld.tile([128, 9, 64], bf16, tag="sg")
            nc.scalar.activation(
                sg, om_T[:, :, 18:27].rearrange("x y k -> x k y"), AF.Sigmoid)
            cym = fld.tile([128, 9, NS, 64], bf16, tag="cym")
            dy_ap = om_T[:, :, 0:18:2].rearrange("x y k -> x k y")
            nc.vector.tensor_tensor(out=cym, in0=bcast(dy_ap, NS, 2),
                                    in1=bcast(syc, 9, 1), op=OP.subtract)
            nc.scalar.activation(cym, cym, AF.Abs)
            nc.vector.tensor_scalar(cym, cym, -1.0, 1.0, OP.mult, OP.add)
            nc.vector.tensor_scalar(cym, cym, 0.0, None, OP.max)
            nc.vector.tensor_tensor(out=cym, in0=cym, in1=bcast(sg, NS, 2),
                                    op=OP.mult)
            cx = fld.tile([128, 9, NS, 64], bf16, tag="cx")
            dx_ap = om_T[:, :, 1:18:2].rearrange("x y k -> x k y")
            nc.vector.tensor_tensor(out=cx, in0=bcast(dx_ap, NS, 2),
                                    in1=bcast(sxc, 9, 1), op=OP.subtract)
            nc.scalar.activation(cx, cx, AF.Abs)
            nc.vector.tensor_scalar(cx, cx, -1.0, 1.0, OP.mult, OP.add)
            nc.vector.tensor_scalar(cx, cx, 0.0, None, OP.max)
            # Bf[x, k, sx, sy, y] = cx * cym (coefficients at the OUTPUT pixel)
            Bf = bfp.tile([128, 9, NS, NS, 64], bf16)
            nc.vector.tensor_tensor(out=Bf, in0=bcast(cx, NS, 3),
                                    in1=bcast(cym, NS, 2), op=OP.mult)
            phase1.close()

            # ---- phase 2: g = per-tap 1x1 conv on padded rows ----
            stackA = contextlib.ExitStack()
            gp = stackA.enter_context(tc.tile_pool(name="gp", bufs=1))
            featp = contextlib.ExitStack()
            fpool = featp.enter_context(tc.tile_pool(name="fpool", bufs=1))
            pg = featp.enter_context(tc.tile_pool(name="pg", bufs=2,
                                                  space="PSUM"))
            feat_sb = fpool.tile([64, 70, 128], bf16)
            nc.sync.dma_start(out=feat_sb, in_=feat_d[0:64, :, :])

            g = gp.tile([128, 9, 64, 70], bf16)
            gf = g.rearrange("x k o y -> x (k o) y")
            for r in range(70):
                psg = pg.tile([128, 2, 512], f32)
                nc.tensor.matmul(psg[:, 0, 0:288], feat_sb[:, r, :],
                                 wk_sb[:, 0:288], start=True, stop=True)
                nc.tensor.matmul(psg[:, 1, 0:288], feat_sb[:, r, :],
                                 wk_sb[:, 288:576], start=True, stop=True)
                cp(gf[:, 0:288, r], psg[:, 0, 0:288])
                cp(gf[:, 288:576, r], psg[:, 1, 0:288])
            featp.close()

            # ---- phase 3: flat warp sum over shifted-g planes ----
            # acc[x, o, y] = sum_{k,sx,sy} Bf[x,k,sx,sy,y]
            #                  * g[x+dlt, k, o, y+ky+sy],  dlt = kx-1+sx.
            # x-shifts of g via partition-shifted SBUF DMAs (per dlt, kx
            # plane group); 5 sy taps fused per op via a sliding-window AP
            # then reduced; o processed in halves to bound tmp size.
            gsp = stackA.enter_context(tc.tile_pool(name="gsp", bufs=1))
            acc = outp.tile([128, 64, 64], bf16)
            Gs = gsp.tile([128, 3, 64, 70], bf16)

            def ywin(ap, off):
                # [.., n(stride 1)] -> [.., 64, 5] sliding window at +off
                new = [list(p) for p in ap.ap[:-1]] + [[1, 64], [1, 5]]
                return bass.AP(tensor=ap.tensor, offset=ap.offset + off,
                               ap=new)

            first = [True, True]

            def warp_terms(slc, delta, kx):
                # slc(ky, o0): [128, 32(o), 70(y')] plane for this kx
                sxi = delta - kx + 3
                for ky in range(3):
                    k = 3 * ky + kx
                    for oh in range(2):
                        o0 = 32 * oh
                        gw = ywin(slc(ky, o0), ky)
                        bf_ap = bcast(Bf[:, k, sxi, :, :], 32, 1).rearrange(
                            "x o s y -> x o y s")
                        tmp = tmpp.tile([128, 32, 64, NS], bf16, tag="t")
                        nc.vector.tensor_tensor(out=tmp, in0=gw, in1=bf_ap,
                                                op=OP.mult)
                        tmp2 = tmpp.tile([128, 32, 64], f32, tag="t2")
                        nc.vector.tensor_reduce(tmp2, tmp,
                                                axis=mybir.AxisListType.X,
                                                op=OP.add)
                        if first[oh]:
                            nc.vector.tensor_copy(acc[:, o0:o0 + 32, :], tmp2)
                            first[oh] = False
                        else:
                            nc.vector.tensor_tensor(
                                out=acc[:, o0:o0 + 32, :],
                                in0=acc[:, o0:o0 + 32, :], in1=tmp2,
                                op=OP.add)

            for kx in range(3):
                warp_terms(
                    lambda ky, o0, kx=kx: g[:, 3 * ky + kx, o0:o0 + 32, :],
                    0, kx)
            for delta in (-3, -2, -1, 1, 2, 3):
                for kx in range(max(0, delta - 1), min(2, delta + 3) + 1):
                    # quadrant-aligned memset band, interior overwritten
                    if delta > 0:
                        nc.vector.memset(Gs[96:128, :, :, :], 0.0)
                        nc.sync.dma_start(
                            out=Gs[0:128 - delta, :, :, :],
                            in_=g[delta:128, kx:9:3, :, :])
                    else:
                        d = -delta
                        nc.vector.memset(Gs[0:32, :, :, :], 0.0)
                        nc.sync.dma_start(
                            out=Gs[d:128, :, :, :],
                            in_=g[0:128 - d, kx:9:3, :, :])
                    warp_terms(
                        lambda ky, o0: Gs[:, ky, o0:o0 + 32, :],
                        delta, kx)

            stackA.close()  # free g + Gs

            # ---- transpose acc -> hacc [(par,y), j, x] ----
            hp = ctx.enter_context(tc.tile_pool(name="hp", bufs=1))
            pv = ctx.enter_context(tc.tile_pool(name="pv", bufs=2,
                                                space="PSUM"))
            pst = ctx.enter_context(tc.tile_pool(name="pst", bufs=1,
                                                 space="PSUM"))
            hacc = hp.tile([128, 32, 128], bf16)
            for j2 in range(4):
                pvt = pv.tile([128, 8, 128], bf16)
                for jj in range(8):
                    j = 8 * j2 + jj
                    nc.tensor.transpose(
                        pvt[:, jj, :],
                        acc[:, 2 * j:2 * j + 2, :].rearrange(
                            "x o y -> x (o y)"),
                        ident)
                cp(hacc[:, 8 * j2:8 * j2 + 8, :], pvt)

            # ---- BN stats ----
            sq = hp.tile([128, 32, 128], bf16, tag="sq")
            nc.vector.tensor_tensor(out=sq, in0=hacc, in1=hacc, op=OP.mult)
            stat2 = fld.tile([128, 2, 32], f32, tag="st2")
            nc.vector.tensor_reduce(stat2[:, 0, :], hacc,
                                    axis=mybir.AxisListType.X, op=OP.add)
            nc.vector.tensor_reduce(stat2[:, 1, :], sq,
                                    axis=mybir.AxisListType.X, op=OP.add)
            ps1 = pst.tile([2, 2, 32], f32)
            nc.tensor.matmul(ps1.rearrange("p a b -> p (a b)"), sp_sb[:, 0:2],
                             stat2.rearrange("p a b -> p (a b)"),
                             start=True, stop=True)
            st_sb = fld.tile([2, 2, 32], f32, tag="stsb")
            nc.vector.tensor_copy(st_sb, ps1)
            cc_in = dram.tile([2, 2, 32], f32)
            cc_out = dram.tile([2, 2, 32], f32)
            nc.sync.dma_start(out=cc_in[:], in_=st_sb)
            nc.gpsimd.collective_compute(
                "AllReduce", OP.add,
                replica_groups=[list(range(N_CORES))],
                ins=[cc_in[:]], outs=[cc_out[:]])
            red = fld.tile([2, 2, 32], f32, tag="red")
            nc.sync.dma_start(out=red, in_=cc_out[:])

            mt = fld.tile([2, 32], f32, tag="mt")
            nc.vector.tensor_scalar(mt, red[:, 0, :], 1.0 / BN_N, None,
                                    OP.mult)
            ex2 = fld.tile([2, 32], f32, tag="ex2")
            nc.vector.tensor_scalar(ex2, red[:, 1, :], 1.0 / BN_N, None,
                                    OP.mult)
            var = fld.tile([2, 32], f32, tag="var")
            nc.vector.tensor_tensor(out=var, in0=mt, in1=mt, op=OP.mult)
            nc.vector.tensor_tensor(out=var, in0=ex2, in1=var, op=OP.subtract)
            nc.vector.tensor_scalar(var, var, EPS, None, OP.add)
            sqv = fld.tile([2, 32], f32, tag="sqv")
            nc.scalar.activation(sqv, var, AF.Sqrt)
            rstd = fld.tile([2, 32], f32, tag="rstd")
            nc.vector.reciprocal(rstd, sqv)
            AB = fld.tile([2, 2, 32], f32, tag="AB")
            nc.vector.tensor_tensor(out=AB[:, 0, :], in0=gb_sb[:, 0, :],
                                    in1=rstd, op=OP.mult)
            nc.vector.tensor_tensor(out=AB[:, 1, :], in0=mt, in1=AB[:, 0, :],
                                    op=OP.mult)
            nc.vector.tensor_tensor(out=AB[:, 1, :], in0=gb_sb[:, 1, :],
                                    in1=AB[:, 1, :], op=OP.subtract)
            ab_d = dram.tile([2, 2, 32], f32)
            nc.sync.dma_start(out=ab_d[:], in_=AB)
            ABc = fld.tile([128, 2, 32], f32, tag="ABc")
            nc.sync.dma_start(
                out=ABc,
                in_=bass.AP(tensor=ab_d.tensor, offset=ab_d.offset,
                            ap=[[64, 2], [0, 64], [32, 2], [1, 32]]))

            # ---- BN apply + store (out = hacc*A + B, one DMA out) ----
            fin = hp.tile([128, 32, 128], bf16)
            nc.vector.tensor_tensor(out=fin, in0=hacc,
                                    in1=bcast(ABc[:, 0, :], 128, 2),
                                    op=OP.mult)
            nc.vector.tensor_tensor(out=fin, in0=fin,
                                    in1=bcast(ABc[:, 1, :], 128, 2),
                                    op=OP.add)
            od = out_d[:]
            out_ap = bass.AP(tensor=od.tensor, offset=od.offset,
                             ap=[[8192, 2], [128, 64], [16384, 32], [1, 128]])
            nc.sync.dma_start(out=out_ap, in_=fin)

    nc.finalize()
    return nc


_module_cache = {}


def get_module():
    if "m" not in _module_cache:
        _module_cache["m"] = build_module()
    return _module_cache["m"]


def prep_inputs(f1_feat, f3_feat, offset_w, offset_b, main_w, gamma, beta):
    """Host-side packing; returns list of 8 in_maps."""
    bf = ml_dtypes.bfloat16
    f1 = np.asarray(f1_feat, np.float32)
    f3 = np.asarray(f3_feat, np.float32)
    ow = np.asarray(offset_w, np.float32)   # [27,128,3,3]
    ob = np.asarray(offset_b, np.float32)
    wk = np.asarray(main_w, np.float32)     # [64,64,3,3]

    # wpack: ow_t [128,243] | wk packed [128,288] | ident [128,128]
    ow_t = ow.reshape(27, 128, 9).transpose(1, 2, 0).reshape(128, 243)
    wk_t = wk.reshape(64, 64, 9).transpose(1, 2, 0).reshape(64, 576)
    wk_r = np.concatenate([wk_t[:, 0:288], wk_t[:, 288:576]], axis=0)
    wpack = np.concatenate(
        [ow_t, wk_r, np.eye(128, dtype=np.float32)], axis=1).astype(bf)

    # spack: sel cols 0-1 | ob col 2 | gb flat col 3
    spack = np.zeros((128, 4), np.float32)
    spack[0:64, 0] = 1.0
    spack[64:128, 1] = 1.0
    spack[0:27, 2] = ob
    gam = np.asarray(gamma, np.float32)
    bet = np.asarray(beta, np.float32)
    gb = np.zeros((2, 2, 32), np.float32)
    for par in range(2):
        gb[par, 0, :] = gam[par::2]
        gb[par, 1, :] = bet[par::2]
    spack[:, 3] = gb.reshape(-1)

    maps = []
    for i in range(N_CORES):
        b, half = i // 2, i % 2
        y0 = 64 * half
        feat = np.zeros((128, 70, 128), np.float32)
        lo, hi = max(0, y0 - 3), min(128, y0 + 67)
        feat[0:64, lo - (y0 - 3):hi - (y0 - 3), :] = f1[b][:, lo:hi, :]
        lo, hi = max(0, y0 - 1), min(128, y0 + 65)
        feat[64:128, lo - (y0 - 1):hi - (y0 - 1), :] = f3[b][:, lo:hi, :]
        maps.append({"feat": feat.astype(bf), "wpack": wpack,
                     "spack": spack})
    return maps


def kernel(**inputs):
    nc = get_module()
    maps = prep_inputs(**inputs)
    res = run_bass_kernel_spmd(nc, maps, core_ids=list(range(N_CORES)))
    out = np.zeros((4, 64, 128, 128), np.float32)
    for i in range(N_CORES):
        b, half = i // 2, i % 2
        out[b, :, 64 * half:64 * half + 64, :] = \
            res.results[i]["out"].astype(np.float32)
    return out


if __name__ == "__main__":
    d = np.load("/root/problem/ref_cache.npz")
    inp = {k: d[k] for k in d.files if k != "expected"}
    got = kernel(**inp)
    exp = d["expected"]
    err = np.linalg.norm(got - exp) / np.linalg.norm(exp)
    print("rel l2 err:", err, "maxabs:", np.abs(got - exp).max())


# revision 28
# speedup vs baseline: 2.1470x; 1.0585x over previous
"""Trainium2 Bass kernel for nn_DeformableAlignment.

Sharding: 8 cores = (batch b in 0..4) x (image row-half in {0,1}).
Each core computes out[b, :, y0:y0+64, :] for y0 = 64*(i%2).

Math (per core, matches reference exactly):
  om  = conv3x3(concat(f1,f3))                          [27, 64, 128]
  dy/dx per tap k; sg = sigmoid(mask-channels)
  bilinear warp written floor-free via hat fields:
    cym[k,sy] = relu(1-|dy-sy|)*sg  (sy in -2..2)       y-coeffs (mask folded)
    cx [k,sx] = relu(1-|dx-sx|)                         x-coeffs
  g[k] = 1x1-conv of f1 with main_w tap k, computed ONCE on the
         y-padded grid: g[x=128 part, (k,o), y70]       (140 matmuls)
  V[k] = sum_sy cym[k,sy] * g[k] shifted in y           (free-dim y offsets)
  out  = sum_k sum_sx cx[k,sx] * V[k] shifted in x      (x-shifts = 6
         partition-shifted SBUF->SBUF DMAs of V)
  BN stats via on-device partial sums + AllReduce across 8 cores.

IO is minimized for the axon tunnel (upload ~94MB/s, download ~40MB/s):
  feat  [128, 70, 128] bf16: parts 0-63 f1 rows y0-3..y0+66,
                             parts 64-127 f3 rows y0-1..y0+64 (+2 pad rows)
  wpack [128, 659] bf16: ow_t [128,243] | wk packed [128,288] | ident
  spack [128, 4] f32: sel | sel | ob | gb-flat
  out   [64, 64, 128] bf16 (converted to f32 on host)
"""

import numpy as np
import ml_dtypes

import concourse.bass as bass
import concourse.bacc as bacc
import concourse.tile as tile
from concourse import mybir
from concourse.bass_utils import run_bass_kernel_spmd

f32 = mybir.dt.float32
bf16 = mybir.dt.bfloat16
AF = mybir.ActivationFunctionType
OP = mybir.AluOpType

N_CORES = 8
NS = 5  # shifts -2..2
EPS = 1e-5
BN_N = 4 * 128 * 128  # elements per channel for batch stats


def bcast(ap, n, dim):
    """Insert a broadcast (step-0) dim of size n at position dim."""
    new = [list(p) for p in ap.ap]
    new.insert(dim, [0, n])
    return bass.AP(tensor=ap.tensor, offset=ap.offset, ap=new)


def build_module():
    nc = bacc.Bacc("TRN2", target_bir_lowering=False, debug=False,
                   num_devices=N_CORES)
    feat_d = nc.dram_tensor("feat", [128, 70, 128], bf16, kind="ExternalInput")
    wp_d = nc.dram_tensor("wpack", [128, 659], bf16, kind="ExternalInput")
    sp_d = nc.dram_tensor("spack", [128, 4], f32, kind="ExternalInput")
    out_d = nc.dram_tensor("out", [64, 64, 128], bf16, kind="ExternalOutput")

    import itertools
    cp_engines = itertools.cycle([0, 1])

    def cp(out, in_):
        if next(cp_engines) == 0:
            nc.vector.tensor_copy(out, in_)
        else:
            nc.scalar.copy(out, in_)

    with tile.TileContext(nc) as tc:
        import contextlib
        ctx = contextlib.ExitStack()
        with ctx:
            const = ctx.enter_context(tc.tile_pool(name="const", bufs=1))
            fld = ctx.enter_context(tc.tile_pool(name="fld", bufs=1))
            dram = ctx.enter_context(tc.tile_pool(name="dram", bufs=1,
                                                  space="DRAM"))
            tmpp = ctx.enter_context(tc.tile_pool(name="tmpp", bufs=1))
            outp = ctx.enter_context(tc.tile_pool(name="outp", bufs=1))
            bfp = ctx.enter_context(tc.tile_pool(name="bfp", bufs=1))

            # ---- constants in ----
            ow_sb = const.tile([128, 9, 27], bf16)
            nc.sync.dma_start(out=ow_sb, in_=wp_d[:, 0:243])
            wk_sb = const.tile([64, 576], bf16)
            nc.sync.dma_start(out=wk_sb[:, 0:288], in_=wp_d[0:64, 243:531])
            nc.sync.dma_start(out=wk_sb[:, 288:576], in_=wp_d[64:128, 243:531])
            ident = const.tile([128, 128], bf16)
            nc.sync.dma_start(out=ident, in_=wp_d[:, 531:659])
            sp_sb = const.tile([128, 4], f32)
            nc.sync.dma_start(out=sp_sb, in_=sp_d[:])
            gb_sb = const.tile([2, 2, 32], f32)
            nc.sync.dma_start(out=gb_sb, in_=sp_d[:, 3:4])
            syc = const.tile([128, NS, 64], bf16)
            sxc = const.tile([128, NS, 64], bf16)
            for i in range(NS):
                nc.vector.memset(syc[:, i, :], float(i - 2))
                nc.vector.memset(sxc[:, i, :], float(i - 2))

            # ---- phase 1: offset conv + fields ----
            phase1 = contextlib.ExitStack()
            xcp = phase1.enter_context(tc.tile_pool(name="xcp", bufs=1))
            fldA = phase1.enter_context(tc.tile_pool(name="fldA", bufs=1))
            omp = phase1.enter_context(tc.tile_pool(name="omp", bufs=2))
            pom = phase1.enter_context(tc.tile_pool(name="pom", bufs=2,
                                                    space="PSUM"))
            ptr = phase1.enter_context(tc.tile_pool(name="ptr", bufs=2,
                                                    space="PSUM"))

            xcat = xcp.tile([128, 66, 130], bf16)
            nc.vector.memset(xcat, 0.0)
            nc.sync.dma_start(out=xcat[0:64, :, 1:129], in_=feat_d[0:64, 2:68, :])
            nc.sync.dma_start(out=xcat[64:128, :, 1:129],
                              in_=feat_d[64:128, 0:66, :])

            om_T = fldA.tile([128, 64, 27], bf16, tag="omT")
            for c in range(16):  # chunks of 4 output rows
                ps = pom.tile([27, 512], f32)
                for k in range(9):
                    ky, kx = k // 3, k % 3
                    rhs = xcat[:, 4 * c + ky:4 * c + ky + 4, kx:kx + 128]
                    nc.tensor.matmul(ps, ow_sb[:, k, :], rhs,
                                     start=(k == 0), stop=(k == 8))
                om_ch = omp.tile([27, 4, 128], bf16)
                nc.vector.tensor_scalar(
                    om_ch, ps.rearrange("p (y x) -> p y x", y=4),
                    sp_sb[0:27, 2:3], None, OP.add)
                pt = ptr.tile([128, 4, 28], bf16)
                for j in range(4):
                    nc.tensor.transpose(pt[:, j, 0:27], om_ch[:, j, :],
                                        ident[0:27, 0:27])
                cp(om_T[:, 4 * c:4 * c + 4, :], pt[:, :, 0:27])

            # fields: cym [x, 9, 5, 64] (mask folded), cx [x, 9, 5, 64]
            sg = fldA.tile([128, 9, 64], bf16, tag="sg")
            nc.scalar.activation(
                sg, om_T[:, :, 18:27].rearrange("x y k -> x k y"), AF.Sigmoid)
            cym = fldA.tile([128, 9, NS, 64], bf16, tag="cym")
            dy_ap = om_T[:, :, 0:18:2].rearrange("x y k -> x k y")
            nc.vector.tensor_tensor(out=cym, in0=bcast(dy_ap, NS, 2),
                                    in1=bcast(syc, 9, 1), op=OP.subtract)
            nc.scalar.activation(cym, cym, AF.Abs)
            nc.vector.tensor_scalar(cym, cym, -1.0, 1.0, OP.mult, OP.add)
            nc.vector.tensor_scalar(cym, cym, 0.0, None, OP.max)
            nc.vector.tensor_tensor(out=cym, in0=cym, in1=bcast(sg, NS, 2),
                                    op=OP.mult)
            cx = fldA.tile([128, 9, NS, 64], bf16, tag="cx")
            dx_ap = om_T[:, :, 1:18:2].rearrange("x y k -> x k y")
            nc.vector.tensor_tensor(out=cx, in0=bcast(dx_ap, NS, 2),
                                    in1=bcast(sxc, 9, 1), op=OP.subtract)
            nc.scalar.activation(cx, cx, AF.Abs)
            nc.vector.tensor_scalar(cx, cx, -1.0, 1.0, OP.mult, OP.add)
            nc.vector.tensor_scalar(cx, cx, 0.0, None, OP.max)
            # Bf[x, k, sx, sy, y] = cx * cym (coefficients at the OUTPUT pixel)
            Bf = bfp.tile([128, 9, NS, NS, 64], bf16)
            nc.vector.tensor_tensor(out=Bf, in0=bcast(cx, NS, 3),
                                    in1=bcast(cym, NS, 2), op=OP.mult)
            phase1.close()

            # ---- phase 2: g = per-tap 1x1 conv on padded rows ----
            stackA = contextlib.ExitStack()
            gp = stackA.enter_context(tc.tile_pool(name="gp", bufs=1))
            featp = contextlib.ExitStack()
            fpool = featp.enter_context(tc.tile_pool(name="fpool", bufs=1))
            pg = featp.enter_context(tc.tile_pool(name="pg", bufs=2,
                                                  space="PSUM"))
            feat_sb = fpool.tile([64, 70, 128], bf16)
            nc.sync.dma_start(out=feat_sb, in_=feat_d[0:64, :, :])

            g = gp.tile([128, 9, 64, 70], bf16)
            ga = g[:]

            def gdst(r):
                # [(2 chunks), 288 (k,o)-cols] view of g[:, :, :, r]
                return bass.AP(tensor=ga.tensor, offset=ga.offset + r,
                               ap=[list(ga.ap[0]), [20160, 2], [70, 288]])

            for r in range(70):
                psg = pg.tile([128, 2, 512], f32)
                nc.tensor.matmul(psg[:, 0, 0:288], feat_sb[:, r, :],
                                 wk_sb[:, 0:288], start=True, stop=True)
                nc.tensor.matmul(psg[:, 1, 0:288], feat_sb[:, r, :],
                                 wk_sb[:, 288:576], start=True, stop=True)
                cp(gdst(r), psg[:, 0:2, 0:288])
            featp.close()

            # ---- phase 3: flat warp sum over shifted-g planes ----
            # acc[x, o, y] = sum_{k,sx,sy} Bf[x,k,sx,sy,y]
            #                  * g[x+dlt, k, o, y+ky+sy],  dlt = kx-1+sx.
            # x-shifts of g via partition-shifted SBUF DMAs (per dlt, kx
            # plane group); 5 sy taps fused per op via a sliding-window AP
            # then reduced; o processed in halves to bound tmp size.
            gsp = stackA.enter_context(tc.tile_pool(name="gsp", bufs=1))
            acc = outp.tile([128, 64, 64], bf16)
            Gs = gsp.tile([128, 3, 64, 70], bf16)

            def ywin(ap, off):
                # [.., n(stride 1)] -> [.., 64, 5] sliding window at +off
                new = [list(p) for p in ap.ap[:-1]] + [[1, 64], [1, 5]]
                return bass.AP(tensor=ap.tensor, offset=ap.offset + off,
                               ap=new)

            first = [True]

            def warp_terms(slc, delta, kx):
                # slc(ky): [128, 64(o), 70(y')] plane for this kx
                sxi = delta - kx + 3
                for ky in range(3):
                    k = 3 * ky + kx
                    gw = ywin(slc(ky), ky)
                    bf_ap = bcast(Bf[:, k, sxi, :, :], 64, 1).rearrange(
                        "x o s y -> x o y s")
                    tmp = tmpp.tile([128, 64, 64, NS], bf16, tag="t")
                    nc.vector.tensor_tensor(out=tmp, in0=gw, in1=bf_ap,
                                            op=OP.mult)
                    tmp2 = tmpp.tile([128, 64, 64], f32, tag="t2")
                    nc.vector.tensor_reduce(tmp2, tmp,
                                            axis=mybir.AxisListType.X,
                                            op=OP.add)
                    if first[0]:
                        nc.vector.tensor_copy(acc, tmp2)
                        first[0] = False
                    else:
                        nc.vector.tensor_tensor(out=acc, in0=acc, in1=tmp2,
                                                op=OP.add)

            for kx in range(3):
                warp_terms(
                    lambda ky, kx=kx: g[:, 3 * ky + kx, :, :], 0, kx)
            for delta in (-3, -2, -1, 1, 2, 3):
                for kx in range(max(0, delta - 1), min(2, delta + 3) + 1):
                    # quadrant-aligned memset band, interior overwritten
                    if delta > 0:
                        nc.vector.memset(Gs[96:128, :, :, :], 0.0)
                        nc.sync.dma_start(
                            out=Gs[0:128 - delta, :, :, :],
                            in_=g[delta:128, kx:9:3, :, :])
                    else:
                        d = -delta
                        nc.vector.memset(Gs[0:32, :, :, :], 0.0)
                        nc.sync.dma_start(
                            out=Gs[d:128, :, :, :],
                            in_=g[0:128 - d, kx:9:3, :, :])
                    warp_terms(lambda ky: Gs[:, ky, :, :], delta, kx)

            stackA.close()  # free g + Gs

            # ---- transpose acc -> hacc [(par,y), j, x] ----
            hp = ctx.enter_context(tc.tile_pool(name="hp", bufs=1))
            pv = ctx.enter_context(tc.tile_pool(name="pv", bufs=2,
                                                space="PSUM"))
            pst = ctx.enter_context(tc.tile_pool(name="pst", bufs=1,
                                                 space="PSUM"))
            hacc = hp.tile([128, 32, 128], bf16)
            for j2 in range(4):
                pvt = pv.tile([128, 8, 128], bf16)
                for jj in range(8):
                    j = 8 * j2 + jj
                    nc.tensor.transpose(
                        pvt[:, jj, :],
                        acc[:, 2 * j:2 * j + 2, :].rearrange(
                            "x o y -> x (o y)"),
                        ident)
                cp(hacc[:, 8 * j2:8 * j2 + 8, :], pvt)

            # ---- BN stats ----
            sq = hp.tile([128, 32, 128], bf16, tag="sq")
            nc.vector.tensor_tensor(out=sq, in0=hacc, in1=hacc, op=OP.mult)
            stat2 = fld.tile([128, 2, 32], f32, tag="st2")
            nc.vector.tensor_reduce(stat2[:, 0, :], hacc,
                                    axis=mybir.AxisListType.X, op=OP.add)
            nc.vector.tensor_reduce(stat2[:, 1, :], sq,
                                    axis=mybir.AxisListType.X, op=OP.add)
            ps1 = pst.tile([2, 2, 32], f32)
            nc.tensor.matmul(ps1.rearrange("p a b -> p (a b)"), sp_sb[:, 0:2],
                             stat2.rearrange("p a b -> p (a b)"),
                             start=True, stop=True)
            st_sb = fld.tile([2, 2, 32], f32, tag="stsb")
            nc.vector.tensor_copy(st_sb, ps1)
            cc_in = dram.tile([2, 2, 32], f32)
            cc_out = dram.tile([2, 2, 32], f32)
            nc.sync.dma_start(out=cc_in[:], in_=st_sb)
            nc.gpsimd.collective_compute(
                "AllReduce", OP.add,
                replica_groups=[list(range(N_CORES))],
                ins=[cc_in[:]], outs=[cc_out[:]])
            red = fld.tile([2, 2, 32], f32, tag="red")
            nc.sync.dma_start(out=red, in_=cc_out[:])

            mt = fld.tile([2, 32], f32, tag="mt")
            nc.vector.tensor_scalar(mt, red[:, 0, :], 1.0 / BN_N, None,
                                    OP.mult)
            ex2 = fld.tile([2, 32], f32, tag="ex2")
            nc.vector.tensor_scalar(ex2, red[:, 1, :], 1.0 / BN_N, None,
                                    OP.mult)
            var = fld.tile([2, 32], f32, tag="var")
            nc.vector.tensor_tensor(out=var, in0=mt, in1=mt, op=OP.mult)
            nc.vector.tensor_tensor(out=var, in0=ex2, in1=var, op=OP.subtract)
            nc.vector.tensor_scalar(var, var, EPS, None, OP.add)
            sqv = fld.tile([2, 32], f32, tag="sqv")
            nc.scalar.activation(sqv, var, AF.Sqrt)
            rstd = fld.tile([2, 32], f32, tag="rstd")
            nc.vector.reciprocal(rstd, sqv)
            AB = fld.tile([2, 2, 32], f32, tag="AB")
            nc.vector.tensor_tensor(out=AB[:, 0, :], in0=gb_sb[:, 0, :],
                                    in1=rstd, op=OP.mult)
            nc.vector.tensor_tensor(out=AB[:, 1, :], in0=mt, in1=AB[:, 0, :],
                                    op=OP.mult)
            nc.vector.tensor_tensor(out=AB[:, 1, :], in0=gb_sb[:, 1, :],
                                    in1=AB[:, 1, :], op=OP.subtract)
            ab_d = dram.tile([2, 2, 32], f32)
            nc.sync.dma_start(out=ab_d[:], in_=AB)
            ABc = fld.tile([128, 2, 32], f32, tag="ABc")
            nc.sync.dma_start(
                out=ABc,
                in_=bass.AP(tensor=ab_d.tensor, offset=ab_d.offset,
                            ap=[[64, 2], [0, 64], [32, 2], [1, 32]]))

            # ---- BN apply + store (out = hacc*A + B, one DMA out) ----
            fin = hp.tile([128, 32, 128], bf16)
            nc.vector.tensor_tensor(out=fin, in0=hacc,
                                    in1=bcast(ABc[:, 0, :], 128, 2),
                                    op=OP.mult)
            nc.vector.tensor_tensor(out=fin, in0=fin,
                                    in1=bcast(ABc[:, 1, :], 128, 2),
                                    op=OP.add)
            od = out_d[:]
            out_ap = bass.AP(tensor=od.tensor, offset=od.offset,
                             ap=[[8192, 2], [128, 64], [16384, 32], [1, 128]])
            nc.sync.dma_start(out=out_ap, in_=fin)

    nc.finalize()
    return nc


_module_cache = {}


def get_module():
    if "m" not in _module_cache:
        _module_cache["m"] = build_module()
    return _module_cache["m"]


def prep_inputs(f1_feat, f3_feat, offset_w, offset_b, main_w, gamma, beta):
    """Host-side packing; returns list of 8 in_maps."""
    bf = ml_dtypes.bfloat16
    f1 = np.asarray(f1_feat, np.float32)
    f3 = np.asarray(f3_feat, np.float32)
    ow = np.asarray(offset_w, np.float32)   # [27,128,3,3]
    ob = np.asarray(offset_b, np.float32)
    wk = np.asarray(main_w, np.float32)     # [64,64,3,3]

    # wpack: ow_t [128,243] | wk packed [128,288] | ident [128,128]
    ow_t = ow.reshape(27, 128, 9).transpose(1, 2, 0).reshape(128, 243)
    wk_t = wk.reshape(64, 64, 9).transpose(1, 2, 0).reshape(64, 576)
    wk_r = np.concatenate([wk_t[:, 0:288], wk_t[:, 288:576]], axis=0)
    wpack = np.concatenate(
        [ow_t, wk_r, np.eye(128, dtype=np.float32)], axis=1).astype(bf)

    # spack: sel cols 0-1 | ob col 2 | gb flat col 3
    spack = np.zeros((128, 4), np.float32)
    spack[0:64, 0] = 1.0
    spack[64:128, 1] = 1.0
    spack[0:27, 2] = ob
    gam = np.asarray(gamma, np.float32)
    bet = np.asarray(beta, np.float32)
    gb = np.zeros((2, 2, 32), np.float32)
    for par in range(2):
        gb[par, 0, :] = gam[par::2]
        gb[par, 1, :] = bet[par::2]
    spack[:, 3] = gb.reshape(-1)

    maps = []
    for i in range(N_CORES):
        b, half = i // 2, i % 2
        y0 = 64 * half
        feat = np.zeros((128, 70, 128), np.float32)
        lo, hi = max(0, y0 - 3), min(128, y0 + 67)
        feat[0:64, lo - (y0 - 3):hi - (y0 - 3), :] = f1[b][:, lo:hi, :]
        lo, hi = max(0, y0 - 1), min(128, y0 + 65)
        feat[64:128, lo - (y0 - 1):hi - (y0 - 1), :] = f3[b][:, lo:hi, :]
        maps.append({"feat": feat.astype(bf), "wpack": wpack,
                     "spack": spack})
    return maps


def kernel(**inputs):
    nc = get_module()
    maps = prep_inputs(**inputs)
    res = run_bass_kernel_spmd(nc, maps, core_ids=list(range(N_CORES)))
    out = np.zeros((4, 64, 128, 128), np.float32)
    for i in range(N_CORES):
        b, half = i // 2, i % 2
        out[b, :, 64 * half:64 * half + 64, :] = \
            res.results[i]["out"].astype(np.float32)
    return out


if __name__ == "__main__":
    d = np.load("/root/problem/ref_cache.npz")
    inp = {k: d[k] for k in d.files if k != "expected"}
    got = kernel(**inp)
    exp = d["expected"]
    err = np.linalg.norm(got - exp) / np.linalg.norm(exp)
    print("rel l2 err:", err, "maxabs:", np.abs(got - exp).max())
